# revision 16
# baseline (speedup 1.0000x reference)
"""Trainium2 Bass kernel for nn_CNN_3496103379215 (normalized conv + argmax pool).

Instruction-count-optimized design (the execution environment is
dispatch-bound: ~47us per matmul instruction, ~33us per DVE op, ~15-22us
per DMA, with no cross-engine overlap — so total time is set by the
instruction stream length, not engine cycles):
  - fp32 matmuls (1 instruction each on TRN2: no InstLdweights, unlike
    fp16 which costs an ldweights+matmult pair) — also bit-accurate, so
    no argmax tie-flip risk and no fp16 split ops.
  - parity-merged Toeplitz T [128, 128] per (channel, dx): even output
    rows land in psum partitions 0..61, odd rows in 64..125 (cols
    62,63,126,127 zero), so one matmul covers both pooling parities and
    all DVE partition windows stay 32-aligned.
  - the 0.25/(wsum+eps) scale is folded into T (both den and nom bands),
    so c1 is the pooled max directly and x1 = nom'/den' is unchanged.
  - both images of the core processed side-by-side in SBUF ([128, 2048]
    tiles) so elementwise work and DMAs are single wide instructions.
  - PSUM: den + nom [128, 2048] (4 banks each) per channel; the 2x2
    argmax pool is 5 wide DVE ops per channel (copy + max + not_equal
    mask + nomin gather via copy_predicated), a 2-DMA parity realign
    (DVE cannot cross partition bases), 3 stage-Y ops and 3 epilogue
    ops per tile.
  - DMAs are plain 2D descriptors per image/channel: a transposed 4D
    DMA is ~1ms in this environment (it decomposes into many
    descriptors), far worse than 4-8 simple DMAs at ~20us each.
"""

import os
import numpy as np
from contextlib import ExitStack

EPS = 1e-20
H = W = 1024
HP = WP = 512
PER_CORE = 2          # images per core
N_CORES = 8
TILE_ROWS = 124       # output rows per y-tile (input rows = 128 incl +-2 halo)
N_TILES = 9           # ceil(1024 / 124)

_CACHE = {}


def _host_tensors(weight, bias):
    """T_all [128, 10*128] fp32 Toeplitz bands (parity-permuted columns:
    col m<62 -> even output row 2m at psum partition m, col 64+j -> odd
    row 2j+1 at partition 64+j; cols 62,63,126,127 zero so DVE partition
    windows stay 32-aligned) and bs [62, 4096]: cols 0:2048 per-(img,ch)
    bias, cols 2048:4096 the 0.25/(wsum+eps) scale."""
    weight = np.asarray(weight, np.float32)
    bias = np.asarray(bias, np.float32)
    wsum = weight.sum(axis=(1, 2, 3))
    sc = (0.25 / (wsum + EPS)).astype(np.float64)
    tall = np.zeros((128, 10 * 128), np.float32)
    for ch in range(2):
        for dx in range(5):
            i = ch * 5 + dx
            T = np.zeros((128, 128), np.float32)
            for m in range(124):
                y = 2 * m if m < 62 else 2 * (m - 62) + 1
                col = m if m < 62 else m + 2
                for dy in range(5):
                    T[y + dy, col] = weight[ch, 0, dy, dx] * sc[ch]
            tall[:, i * 128:(i + 1) * 128] = T
    bs = np.zeros((62, 2048), np.float32)
    for img in range(2):
        for ch in range(2):
            c0 = img * 1024 + ch * 512
            bs[:, c0:c0 + 512] = bias[ch]
    return tall, bs


def _build_program(repeat=1):
    import concourse.bass as bass
    import concourse.tile as tile
    from concourse import bacc, mybir

    # diagnostic: build only the matmul/dc/in-DMA stream (outputs garbage)
    skip_pool = bool(int(os.environ.get("BASS_SKIP_POOL", "0")))

    f32 = mybir.dt.float32
    u8 = mybir.dt.uint8
    nc = bacc.Bacc("TRN2", target_bir_lowering=False)

    data_ext = nc.declare_dram_parameter("data", [PER_CORE, 1, H, W], f32, isOutput=False)
    conf_ext = nc.declare_dram_parameter("conf", [PER_CORE, 1, H, W], f32, isOutput=False)
    tall_ext = nc.declare_dram_parameter("tall", [128, 10 * 128], f32, isOutput=False)
    bs_ext = nc.declare_dram_parameter("bs", [62, 2048], f32, isOutput=False)
    x1_ext = nc.declare_dram_parameter("x1", [PER_CORE, 2, HP, WP], f32, isOutput=True)
    c1_ext = nc.declare_dram_parameter("c1", [PER_CORE, 2, HP, WP], f32, isOutput=True)

    gt = mybir.AluOpType.is_gt
    ne = mybir.AluOpType.not_equal
    mx = mybir.AluOpType.max
    pmax = mybir.PoolFunctionType.max

    with tile.TileContext(nc) as tc, ExitStack() as ctx:
        consts = ctx.enter_context(tc.tile_pool(name="consts", bufs=1))
        inp = ctx.enter_context(tc.tile_pool(name="inp", bufs=2))
        psum = ctx.enter_context(tc.tile_pool(name="psum", bufs=1, space="PSUM"))
        sx = ctx.enter_context(tc.tile_pool(name="sx", bufs=2))

        tall_t = consts.tile([128, 10 * 128], f32)
        nc.sync.dma_start(out=tall_t[:, :], in_=tall_ext[:, :])
        bs_t = consts.tile([62, 2048], f32)
        nc.sync.dma_start(out=bs_t[:, :], in_=bs_ext[:, :])

        for _rep in range(repeat):
          for t in range(N_TILES):
            ys = TILE_ROWS * t
            r0 = ys - 2                      # first input row of tile (may be <0)
            cr0, cr1 = max(r0, 0), min(r0 + 128, H)
            p0, p1 = cr0 - r0, cr1 - r0

            conf2 = inp.tile([128, 2048], f32, tag="conf2")
            data2 = inp.tile([128, 2048], f32, tag="data2")
            # zero y-halo rows at image top/bottom BEFORE the load
            # (32-aligned partition window; DMA overwrites valid rows)
            for tt in (conf2, data2):
                if p0 > 0:
                    nc.vector.memset(tt[0:32, :], 0.0)
                if p1 < 128:
                    pb = p1 // 32 * 32
                    nc.vector.memset(tt[pb:pb + 32, :], 0.0)
            for img in range(2):
                nc.sync.dma_start(out=conf2[p0:p1, img * 1024:(img + 1) * 1024],
                                  in_=conf_ext[img, 0, cr0:cr1, :])
                nc.sync.dma_start(out=data2[p0:p1, img * 1024:(img + 1) * 1024],
                                  in_=data_ext[img, 0, cr0:cr1, :])

            dc2 = inp.tile([128, 2048], f32, tag="dc2")
            nc.vector.tensor_mul(dc2[:, :], data2[:, :], conf2[:, :])

            # pooled-stage tiles: col blocks b = img*2 + ch, 512 wide
            cx = sx.tile([128, 2048], f32, tag="cx")
            nx = sx.tile([128, 2048], f32, tag="nx")
            mk = sx.tile([128, 2048], u8, tag="mk")

            for ch in range(2):
                den = psum.tile([128, 2048], f32, tag="den")
                nom = psum.tile([128, 2048], f32, tag="nom")
                for k, dx in enumerate((2, 0, 1, 3, 4)):
                    jA = max(0, 2 - dx)          # chunk A valid out cols [jA, 512)
                    jB = min(512, 514 - dx)      # chunk B valid out cols [0, jB)
                    TT = tall_t[:, (ch * 5 + dx) * 128:(ch * 5 + dx + 1) * 128]
                    st, sp = k == 0, k == 4
                    for bank, src in ((den, conf2), (nom, dc2)):
                        for img in range(2):
                            b = img * 1024
                            nc.tensor.matmul(bank[0:128, b + jA:b + 512], TT,
                                             src[:, b + jA + dx - 2:b + 510 + dx],
                                             start=st, stop=sp)
                            nc.tensor.matmul(bank[0:128, b + 512:b + 512 + jB], TT,
                                             src[:, b + 510 + dx:b + 510 + dx + jB],
                                             start=st, stop=sp)

                if skip_pool:
                    continue
                # ---- stage X: pool x-pairs; mask = (even != max) i.e. odd
                # strictly greater (first-wins ties); gather nomin ----
                d4 = den[0:128, :].rearrange("p (i x two) -> p i x two", i=2, two=2)
                n4 = nom[0:128, :].rearrange("p (i x two) -> p i x two", i=2, two=2)
                cxv = cx[0:128, :].rearrange("p (i r) -> p i r", i=2)[:, :, ch * 512:(ch + 1) * 512]
                nxv = nx[0:128, :].rearrange("p (i r) -> p i r", i=2)[:, :, ch * 512:(ch + 1) * 512]
                mkv = mk[0:128, :].rearrange("p (i r) -> p i r", i=2)[:, :, ch * 512:(ch + 1) * 512]
                nc.vector.tensor_copy(cxv, d4[:, :, :, 0])
                nc.vector.tensor_tensor(cxv, cxv, d4[:, :, :, 1], op=mx)
                nc.vector.tensor_tensor(mkv, d4[:, :, :, 0], cxv, op=ne)
                nc.vector.tensor_copy(nxv, n4[:, :, :, 0])
                nc.vector.copy_predicated(nxv, mkv, n4[:, :, :, 1])

            if skip_pool:
                continue
            # ---- stage Y: odd row beats even row only if strictly greater.
            # DVE ops need equal SBUF base partitions, so first realign the
            # odd-parity rows (partitions 64..125) to base 0 via on-chip DMA.
            cxo = sx.tile([128, 2048], f32, tag="cxo")
            nxo = sx.tile([128, 2048], f32, tag="nxo")
            nc.sync.dma_start(out=cxo[0:62, :], in_=cx[64:126, :])
            nc.sync.dma_start(out=nxo[0:62, :], in_=nx[64:126, :])
            my = sx.tile([128, 2048], u8, tag="my")
            nc.vector.tensor_tensor(my[0:62, :], cxo[0:62, :], cx[0:62, :], op=gt)
            nc.vector.tensor_tensor(cx[0:62, :], cx[0:62, :], cxo[0:62, :], op=mx)
            nc.vector.copy_predicated(nx[0:62, :], my[0:62, :], nxo[0:62, :])

            # ---- epilogue: x1 = nx/cx + bias ; c1 = cx * sc ----
            rv = sx.tile([128, 2048], f32, tag="rv")
            nc.vector.reciprocal_approx_fast(rv[0:62, :], cx[0:62, :])
            nc.vector.tensor_mul(nx[0:62, :], nx[0:62, :], rv[0:62, :])
            nc.vector.tensor_add(nx[0:62, :], nx[0:62, :], bs_t[:, :])

            n_valid = min(TILE_ROWS, H - ys)
            npool = (n_valid + 1) // 2
            pr0 = ys // 2
            for ext, tsrc in ((x1_ext, nx), (c1_ext, cx)):
                for img in range(2):
                    for ch in range(2):
                        cb = (img * 2 + ch) * 512
                        nc.sync.dma_start(
                            out=ext[img, ch, pr0:pr0 + npool, :],
                            in_=tsrc[0:npool, cb:cb + 512])
    nc.compile()
    return nc


def kernel(data, conf, weight, bias):
    from concourse.bass_utils import run_bass_kernel_spmd

    data = np.ascontiguousarray(np.asarray(data, np.float32))
    conf = np.ascontiguousarray(np.asarray(conf, np.float32))
    repeat = int(os.environ.get("BASS_KERNEL_REPEAT", "1"))
    key = ("nc", repeat, os.environ.get("BASS_SKIP_POOL", "0"))
    if key not in _CACHE:
        _CACHE[key] = _build_program(repeat)
    nc = _CACHE[key]

    tall, bs = _host_tensors(weight, bias)
    in_maps = []
    for c in range(N_CORES):
        sl = slice(c * PER_CORE, (c + 1) * PER_CORE)
        in_maps.append({"data": data[sl], "conf": conf[sl],
                        "tall": tall, "bs": bs})

    trace = bool(int(os.environ.get("BASS_KERNEL_TRACE", "0")))
    res = run_bass_kernel_spmd(nc, in_maps, list(range(N_CORES)), trace=trace)
    kernel.last_exec_time_ns = res.exec_time_ns

    x1 = np.concatenate([r["x1"] for r in res.results], axis=0)
    c1 = np.concatenate([r["c1"] for r in res.results], axis=0)
    return x1, c1


kernel.last_exec_time_ns = None
